# revision 31
# baseline (speedup 1.0000x reference)
"""Trainium2 Bass kernel for nn_Block2x2DiagProductRectangular.

The reference applies 10 butterfly stages (fixed 2x2 factor matrices) along the
feature axis of x [16384, 1024], then adds a bias. Since the factors are fixed
inputs, the whole chain is one dense linear map: out = x @ M + bias with
M = product of the butterfly stage matrices (1024x1024).

Strategy:
  - Host: build M in float64 from abcd_list, cast to fp32.
  - Shard batch across 8 NeuronCores (2048 rows each).
  - Host pre-transposes each x shard so the device needs no on-chip transposes:
    the PE matmul stationary operand is x^T tiles [K=128 feat, M=128 batch],
    moving operand is M row-blocks [128, 512] resident in SBUF, accumulating
    out tiles [128 batch, 1024 feat] in PSUM over 8 K-tiles (fp32r, 1 cyc/row).
  - PSUM accumulators are bank-sized [128, 512]; DVE drains them with a fused
    bias add into SBUF, stores leave on the second HWDGE queue (nc.scalar).
  - Group 0's inputs arrive as per-K chunks (first matmul waits for ~0.75 MB)
    and its compute loop runs K outermost so PE consumes chunks as they land;
    later groups are prefetched, split across both HWDGE queues.
  - Dummy matmuls warm the PE HAM clock gate during the initial load window.
"""

import numpy as np

import concourse.bass as bass
import concourse.mybir as mybir
import concourse.tile as tile
from concourse import bacc
from concourse.bass_utils import run_bass_kernel_spmd

BATCH = 16384
N = 1024
P = 128
NCORES = 8
ROWS_PER_CORE = BATCH // NCORES          # 2048
GROUPS = 4                               # batch groups per core (512 rows each)
GROUP_ROWS = ROWS_PER_CORE // GROUPS     # 512
BT_PER_GROUP = GROUP_ROWS // P           # 4
KO = N // P                              # 8 k-tiles
NH = N // 512                            # 2 psum-bank halves
WARMUP_MM = 8


def _build_dense_matrix(abcd_list):
    """Dense M (float64) such that reference(x) == x @ M + bias."""
    out = np.eye(N, dtype=np.float64)
    for abcd in abcd_list[::-1]:
        half = abcd.shape[-1]
        a = np.asarray(abcd, dtype=np.float64)[0]          # [2, 2, half]
        y = out.reshape(N, -1, 2, half)
        y = np.einsum('ikj,bgkj->bgij', a, y)
        out = y.reshape(N, N)
    return out


def _build_bass():
    nc = bacc.Bacc(None, target_bir_lowering=False, debug=False)
    xt_d = nc.dram_tensor(
        "xt", (GROUPS, P, KO, GROUP_ROWS), mybir.dt.float32r, kind="ExternalInput"
    )
    wt_d = nc.dram_tensor("wt", (KO, P, N), mybir.dt.float32r, kind="ExternalInput")
    bias_d = nc.dram_tensor("bias_bc", (1, N), mybir.dt.float32, kind="ExternalInput")
    out_d = nc.dram_tensor(
        "out", (ROWS_PER_CORE, N), mybir.dt.float32, kind="ExternalOutput"
    )

    with tile.TileContext(nc) as tc:
        with (
            tc.tile_pool(name="const", bufs=1) as const_pool,
            tc.tile_pool(name="xt", bufs=4) as xt_pool,
            tc.tile_pool(name="outs", bufs=8) as out_pool,
            tc.tile_pool(name="psum", bufs=8, space="PSUM") as psum_pool,
        ):
            warm_sb = const_pool.tile([P, 512], mybir.dt.float32)
            nc.gpsimd.memset(warm_sb[:], 0.0)

            wt_sb = const_pool.tile([P, KO, N], mybir.dt.float32r)
            bias_sb = const_pool.tile([P, N], mybir.dt.float32)

            xt_tiles = []
            for g in range(GROUPS):
                xt_sb = xt_pool.tile([P, KO, GROUP_ROWS], mybir.dt.float32r,
                                     name=f"xt_sb_{g}", tag="xt_sb")
                xt_tiles.append(xt_sb)
            # Group 0 chunked with the W tiles so the first matmul starts
            # early; group 1 also chunked so its data drips in during the
            # group-0 tail. All loads stay on one queue in need order so
            # nothing steals HBM bandwidth from earlier-needed bytes.
            for ko in range(KO):
                nc.sync.dma_start(wt_sb[:, ko, :512], wt_d[ko][:, :512])
                nc.sync.dma_start(xt_tiles[0][:, ko, :], xt_d[0][:, ko, :])
                nc.sync.dma_start(wt_sb[:, ko, 512:], wt_d[ko][:, 512:])
            # bias arrives as one 4 KB row and is broadcast on-device by
            # doubling SBUF->SBUF copies on the otherwise idle SWDGE queue,
            # keeping 508 KB off the supply-critical HBM read stream.
            nc.gpsimd.dma_start(bias_sb[0:1, :], bias_d[:])
            rows = 1
            while rows < P:
                nc.gpsimd.dma_start(
                    bias_sb[rows:2 * rows, :], bias_sb[0:rows, :]
                )
                rows *= 2
            for ko in range(KO):
                nc.sync.dma_start(xt_tiles[1][:, ko, :], xt_d[1][:, ko, :])
            for ko in range(0, KO, 2):
                nc.sync.dma_start(xt_tiles[2][:, ko:ko + 2, :],
                                  xt_d[2][:, ko:ko + 2, :])
            for ko in range(0, KO, 2):
                nc.sync.dma_start(xt_tiles[3][:, ko:ko + 2, :],
                                  xt_d[3][:, ko:ko + 2, :])

            def alloc_acc(name):
                return [
                    psum_pool.tile([P, 512], mybir.dt.float32, name=f"{name}_{h}",
                                   tag="ps_acc")
                    for h in range(NH)
                ]

            def mm(acc, xt_sb, bt, ko):
                lhsT = xt_sb[:, ko, bt * P:(bt + 1) * P]
                for h in range(NH):
                    nc.tensor.matmul(
                        acc[h][:],
                        lhsT,
                        wt_sb[:, ko, h * 512:(h + 1) * 512],
                        start=(ko == 0),
                        stop=(ko == KO - 1),
                    )

            def drain(g, bt, acc, split_store=False):
                out_sb = out_pool.tile([P, N], mybir.dt.float32, name="out_sb")
                row0 = g * GROUP_ROWS + bt * P
                for h in range(NH):
                    nc.vector.tensor_add(
                        out=out_sb[:, h * 512:(h + 1) * 512],
                        in0=acc[h][:],
                        in1=bias_sb[:, h * 512:(h + 1) * 512],
                    )
                    if split_store:
                        # store each half as soon as its drain lands so the
                        # final transfer isn't a full 512 KB on the critical
                        # path (only worth the extra issue cost at the tail)
                        nc.scalar.dma_start(
                            out_d[row0:row0 + P, h * 512:(h + 1) * 512],
                            out_sb[:, h * 512:(h + 1) * 512],
                        )
                if not split_store:
                    nc.scalar.dma_start(out_d[row0:row0 + P, :], out_sb[:])

            # Group 0: K outermost across all 4 batch tiles so each arriving
            # (wt, xt) chunk is consumed by 8 matmuls while later chunks load.
            accs0 = [alloc_acc(f"ps_g0_b{bt}") for bt in range(BT_PER_GROUP)]

            # PE warm-up: dummy matmuls on scratch data (no DMA dependency) so
            # the HAM clock gate opens during the initial load window. They
            # scribble on group 0 / bt 0's accumulator, which the real ko=0
            # matmul resets via start=True.
            for _ in range(WARMUP_MM):
                nc.tensor.matmul(
                    accs0[0][0][:, :P], warm_sb[:, :P], warm_sb[:, :P],
                    start=True, stop=True,
                )

            for ko in range(KO):
                for bt in range(BT_PER_GROUP):
                    mm(accs0[bt], xt_tiles[0], bt, ko)
                    if ko == KO - 1:
                        # drain as soon as this tile's accumulation closes so
                        # PSUM slots free up for the next group
                        drain(0, bt, accs0[bt])

            # Group 1: same K-outer shape — consumes its chunks as they land.
            accs1 = [alloc_acc(f"ps_g1_b{bt}") for bt in range(BT_PER_GROUP)]
            for ko in range(KO):
                for bt in range(BT_PER_GROUP):
                    mm(accs1[bt], xt_tiles[1], bt, ko)
                    if ko == KO - 1:
                        drain(1, bt, accs1[bt])

            # Groups 2-3: data prefetched; accumulate per batch tile.
            for g in range(2, GROUPS):
                for bt in range(BT_PER_GROUP):
                    acc = alloc_acc("ps_acc")
                    for ko in range(KO):
                        mm(acc, xt_tiles[g], bt, ko)
                    last = (g == GROUPS - 1) and (bt == BT_PER_GROUP - 1)
                    drain(g, bt, acc, split_store=last)

    nc.compile()
    return nc


def kernel(x, abcd_list, bias, _trace=False):
    x = np.ascontiguousarray(np.asarray(x, dtype=np.float32))
    bias = np.asarray(bias, dtype=np.float32)

    M = _build_dense_matrix(abcd_list).astype(np.float32)
    wt3 = np.ascontiguousarray(M.reshape(KO, P, N))       # [ko, p, n]
    bias_bc = np.ascontiguousarray(bias[None, :])

    nc = _build_bass()

    in_maps = []
    for c in range(NCORES):
        xs = x[c * ROWS_PER_CORE:(c + 1) * ROWS_PER_CORE]
        # xt4[g, p, ko, b] = xs[g*512 + b, ko*128 + p]
        xt4 = np.ascontiguousarray(
            xs.reshape(GROUPS, GROUP_ROWS, KO, P).transpose(0, 3, 2, 1)
        )
        in_maps.append({"xt": xt4, "wt": wt3, "bias_bc": bias_bc})

    res = run_bass_kernel_spmd(
        nc, in_maps, core_ids=list(range(NCORES)), trace=_trace
    )
    out = np.concatenate([r["out"] for r in res.results], axis=0)
    if _trace:
        kernel.last_results = res
    return out


# revision 32
# speedup vs baseline: 1.3168x; 1.3168x over previous
"""Trainium2 Bass kernel for nn_Block2x2DiagProductRectangular.

The reference applies 10 butterfly stages (fixed 2x2 factor matrices) along the
feature axis of x [16384, 1024], then adds a bias. Since the factors are fixed
inputs, the whole chain is one dense linear map: out = x @ M + bias with
M = product of the butterfly stage matrices (1024x1024).

Strategy:
  - Host: build M in float64 from abcd_list, cast to fp32.
  - Shard batch across 8 NeuronCores (2048 rows each).
  - Host pre-transposes each x shard so the device needs no on-chip transposes:
    the PE matmul stationary operand is x^T tiles [K=128 feat, M=128 batch],
    moving operand is M row-blocks [128, 512] resident in SBUF, accumulating
    out tiles [128 batch, 1024 feat] in PSUM over 8 K-tiles (fp32r, 1 cyc/row).
  - PSUM accumulators are bank-sized [128, 512]; DVE drains them with a fused
    bias add into SBUF, stores leave on the second HWDGE queue (nc.scalar).
  - Group 0's inputs arrive as per-K chunks (first matmul waits for ~0.75 MB)
    and its compute loop runs K outermost so PE consumes chunks as they land;
    later groups are prefetched, split across both HWDGE queues.
  - Dummy matmuls warm the PE HAM clock gate during the initial load window.
"""

import numpy as np

import concourse.bass as bass
import concourse.mybir as mybir
import concourse.tile as tile
from concourse import bacc
from concourse.bass_utils import run_bass_kernel_spmd

BATCH = 16384
N = 1024
P = 128
NCORES = 8
ROWS_PER_CORE = BATCH // NCORES          # 2048
GROUPS = 4                               # batch groups per core (512 rows each)
GROUP_ROWS = ROWS_PER_CORE // GROUPS     # 512
BT_PER_GROUP = GROUP_ROWS // P           # 4
KO = N // P                              # 8 k-tiles
NH = N // 512                            # 2 psum-bank halves
WARMUP_MM = 8


def _build_dense_matrix(abcd_list):
    """Dense M (float64) such that reference(x) == x @ M + bias."""
    out = np.eye(N, dtype=np.float64)
    for abcd in abcd_list[::-1]:
        half = abcd.shape[-1]
        a = np.asarray(abcd, dtype=np.float64)[0]          # [2, 2, half]
        y = out.reshape(N, -1, 2, half)
        y = np.einsum('ikj,bgkj->bgij', a, y)
        out = y.reshape(N, N)
    return out


def _build_bass():
    nc = bacc.Bacc(None, target_bir_lowering=False, debug=False)
    xt_d = nc.dram_tensor(
        "xt", (GROUPS, P, KO, GROUP_ROWS), mybir.dt.float32r, kind="ExternalInput"
    )
    wt_d = nc.dram_tensor("wt", (KO, P, N), mybir.dt.float32r, kind="ExternalInput")
    bias_d = nc.dram_tensor("bias_bc", (1, N), mybir.dt.float32, kind="ExternalInput")
    out_d = nc.dram_tensor(
        "out", (ROWS_PER_CORE, N), mybir.dt.float32, kind="ExternalOutput"
    )

    with tile.TileContext(nc) as tc:
        with (
            tc.tile_pool(name="const", bufs=1) as const_pool,
            tc.tile_pool(name="xt", bufs=4) as xt_pool,
            tc.tile_pool(name="outs", bufs=8) as out_pool,
            tc.tile_pool(name="psum", bufs=8, space="PSUM") as psum_pool,
        ):
            warm_sb = const_pool.tile([P, 512], mybir.dt.float32)
            nc.gpsimd.memset(warm_sb[:], 0.0)

            wt_sb = const_pool.tile([P, KO, N], mybir.dt.float32r)
            bias_sb = const_pool.tile([P, N], mybir.dt.float32)

            xt_tiles = []
            for g in range(GROUPS):
                xt_sb = xt_pool.tile([P, KO, GROUP_ROWS], mybir.dt.float32r,
                                     name=f"xt_sb_{g}", tag="xt_sb")
                xt_tiles.append(xt_sb)
            # Group 0 chunked with the W tiles so the first matmul starts
            # early; group 1 also chunked so its data drips in during the
            # group-0 tail. All loads stay on one queue in need order so
            # nothing steals HBM bandwidth from earlier-needed bytes.
            for ko in range(KO):
                nc.sync.dma_start(wt_sb[:, ko, :512], wt_d[ko][:, :512])
                nc.sync.dma_start(xt_tiles[0][:, ko, :], xt_d[0][:, ko, :])
                nc.sync.dma_start(wt_sb[:, ko, 512:], wt_d[ko][:, 512:])
            # bias arrives as one 4 KB row and is replicated across partitions
            # by a single broadcast-AP DMA on the otherwise idle SWDGE queue,
            # keeping 508 KB off the supply-critical HBM read stream.
            nc.gpsimd.dma_start(
                bias_sb[:], bias_d[0:1, :].to_broadcast((P, N))
            )
            for ko in range(KO):
                nc.sync.dma_start(xt_tiles[1][:, ko, :], xt_d[1][:, ko, :])
            for ko in range(0, KO, 2):
                nc.sync.dma_start(xt_tiles[2][:, ko:ko + 2, :],
                                  xt_d[2][:, ko:ko + 2, :])
            for ko in range(0, KO, 2):
                nc.sync.dma_start(xt_tiles[3][:, ko:ko + 2, :],
                                  xt_d[3][:, ko:ko + 2, :])

            def alloc_acc(name):
                return [
                    psum_pool.tile([P, 512], mybir.dt.float32, name=f"{name}_{h}",
                                   tag="ps_acc")
                    for h in range(NH)
                ]

            def mm(acc, xt_sb, bt, ko):
                lhsT = xt_sb[:, ko, bt * P:(bt + 1) * P]
                for h in range(NH):
                    nc.tensor.matmul(
                        acc[h][:],
                        lhsT,
                        wt_sb[:, ko, h * 512:(h + 1) * 512],
                        start=(ko == 0),
                        stop=(ko == KO - 1),
                    )

            def drain(g, bt, acc, split_store=False):
                out_sb = out_pool.tile([P, N], mybir.dt.float32, name="out_sb")
                row0 = g * GROUP_ROWS + bt * P
                for h in range(NH):
                    nc.vector.tensor_add(
                        out=out_sb[:, h * 512:(h + 1) * 512],
                        in0=acc[h][:],
                        in1=bias_sb[:, h * 512:(h + 1) * 512],
                    )
                    if split_store:
                        # store each half as soon as its drain lands so the
                        # final transfer isn't a full 512 KB on the critical
                        # path (only worth the extra issue cost at the tail)
                        nc.scalar.dma_start(
                            out_d[row0:row0 + P, h * 512:(h + 1) * 512],
                            out_sb[:, h * 512:(h + 1) * 512],
                        )
                if not split_store:
                    nc.scalar.dma_start(out_d[row0:row0 + P, :], out_sb[:])

            # Group 0: K outermost across all 4 batch tiles so each arriving
            # (wt, xt) chunk is consumed by 8 matmuls while later chunks load.
            accs0 = [alloc_acc(f"ps_g0_b{bt}") for bt in range(BT_PER_GROUP)]

            # PE warm-up: dummy matmuls on scratch data (no DMA dependency) so
            # the HAM clock gate opens during the initial load window. They
            # scribble on group 0 / bt 0's accumulator, which the real ko=0
            # matmul resets via start=True.
            for _ in range(WARMUP_MM):
                nc.tensor.matmul(
                    accs0[0][0][:, :P], warm_sb[:, :P], warm_sb[:, :P],
                    start=True, stop=True,
                )

            for ko in range(KO):
                for bt in range(BT_PER_GROUP):
                    mm(accs0[bt], xt_tiles[0], bt, ko)
                    if ko == KO - 1:
                        # drain as soon as this tile's accumulation closes so
                        # PSUM slots free up for the next group
                        drain(0, bt, accs0[bt])

            # Group 1: same K-outer shape — consumes its chunks as they land.
            accs1 = [alloc_acc(f"ps_g1_b{bt}") for bt in range(BT_PER_GROUP)]
            for ko in range(KO):
                for bt in range(BT_PER_GROUP):
                    mm(accs1[bt], xt_tiles[1], bt, ko)
                    if ko == KO - 1:
                        drain(1, bt, accs1[bt])

            # Groups 2-3: data prefetched; accumulate per batch tile.
            for g in range(2, GROUPS):
                for bt in range(BT_PER_GROUP):
                    acc = alloc_acc("ps_acc")
                    for ko in range(KO):
                        mm(acc, xt_tiles[g], bt, ko)
                    last = (g == GROUPS - 1) and (bt == BT_PER_GROUP - 1)
                    drain(g, bt, acc, split_store=last)

    nc.compile()
    return nc


def kernel(x, abcd_list, bias, _trace=False):
    x = np.ascontiguousarray(np.asarray(x, dtype=np.float32))
    bias = np.asarray(bias, dtype=np.float32)

    M = _build_dense_matrix(abcd_list).astype(np.float32)
    wt3 = np.ascontiguousarray(M.reshape(KO, P, N))       # [ko, p, n]
    bias_bc = np.ascontiguousarray(bias[None, :])

    nc = _build_bass()

    in_maps = []
    for c in range(NCORES):
        xs = x[c * ROWS_PER_CORE:(c + 1) * ROWS_PER_CORE]
        # xt4[g, p, ko, b] = xs[g*512 + b, ko*128 + p]
        xt4 = np.ascontiguousarray(
            xs.reshape(GROUPS, GROUP_ROWS, KO, P).transpose(0, 3, 2, 1)
        )
        in_maps.append({"xt": xt4, "wt": wt3, "bias_bc": bias_bc})

    res = run_bass_kernel_spmd(
        nc, in_maps, core_ids=list(range(NCORES)), trace=_trace
    )
    out = np.concatenate([r["out"] for r in res.results], axis=0)
    if _trace:
        kernel.last_results = res
    return out


# revision 33
# speedup vs baseline: 1.3906x; 1.0561x over previous
"""Trainium2 Bass kernel for nn_Block2x2DiagProductRectangular.

The reference applies 10 butterfly stages (fixed 2x2 factor matrices) along the
feature axis of x [16384, 1024], then adds a bias. Since the factors are fixed
inputs, the whole chain is one dense linear map: out = x @ M + bias with
M = product of the butterfly stage matrices (1024x1024).

Strategy:
  - Host: build M in float64 from abcd_list, cast to fp32.
  - Shard batch across 8 NeuronCores (2048 rows each).
  - Host pre-transposes each x shard so the device needs no on-chip transposes:
    the PE matmul stationary operand is x^T tiles [K=128 feat, M=128 batch],
    moving operand is M row-blocks [128, 512] resident in SBUF, accumulating
    out tiles [128 batch, 1024 feat] in PSUM over 8 K-tiles (fp32r, 1 cyc/row).
  - PSUM accumulators are bank-sized [128, 512]; DVE drains them with a fused
    bias add into SBUF, stores leave on the second HWDGE queue (nc.scalar).
  - Group 0's inputs arrive as per-K chunks (first matmul waits for ~0.75 MB)
    and its compute loop runs K outermost so PE consumes chunks as they land;
    later groups are prefetched, split across both HWDGE queues.
  - Dummy matmuls warm the PE HAM clock gate during the initial load window.
"""

import numpy as np

import concourse.bass as bass
import concourse.mybir as mybir
import concourse.tile as tile
from concourse import bacc
from concourse.bass_utils import run_bass_kernel_spmd

BATCH = 16384
N = 1024
P = 128
NCORES = 8
ROWS_PER_CORE = BATCH // NCORES          # 2048
GROUPS = 4                               # batch groups per core (512 rows each)
GROUP_ROWS = ROWS_PER_CORE // GROUPS     # 512
BT_PER_GROUP = GROUP_ROWS // P           # 4
KO = N // P                              # 8 k-tiles
NH = N // 512                            # 2 psum-bank halves
WARMUP_MM = 8


def _build_dense_matrix(abcd_list):
    """Dense M (float64) such that reference(x) == x @ M + bias."""
    out = np.eye(N, dtype=np.float64)
    for abcd in abcd_list[::-1]:
        half = abcd.shape[-1]
        a = np.asarray(abcd, dtype=np.float64)[0]          # [2, 2, half]
        y = out.reshape(N, -1, 2, half)
        y = np.einsum('ikj,bgkj->bgij', a, y)
        out = y.reshape(N, N)
    return out


def _build_bass():
    nc = bacc.Bacc(None, target_bir_lowering=False, debug=False)
    xt_d = nc.dram_tensor(
        "xt", (GROUPS, P, KO, GROUP_ROWS), mybir.dt.float32r, kind="ExternalInput"
    )
    wt_d = nc.dram_tensor("wt", (KO, P, N), mybir.dt.float32r, kind="ExternalInput")
    bias_d = nc.dram_tensor("bias_bc", (P, N), mybir.dt.float32, kind="ExternalInput")
    out_d = nc.dram_tensor(
        "out", (ROWS_PER_CORE, N), mybir.dt.float32, kind="ExternalOutput"
    )

    with tile.TileContext(nc) as tc:
        with (
            tc.tile_pool(name="const", bufs=1) as const_pool,
            tc.tile_pool(name="xt", bufs=4) as xt_pool,
            tc.tile_pool(name="outs", bufs=8) as out_pool,
            tc.tile_pool(name="psum", bufs=8, space="PSUM") as psum_pool,
        ):
            warm_sb = const_pool.tile([P, 512], mybir.dt.float32)
            nc.gpsimd.memset(warm_sb[:], 0.0)

            wt_sb = const_pool.tile([P, KO, N], mybir.dt.float32r)
            bias_sb = const_pool.tile([P, N], mybir.dt.float32)

            xt_tiles = []
            for g in range(GROUPS):
                xt_sb = xt_pool.tile([P, KO, GROUP_ROWS], mybir.dt.float32r,
                                     name=f"xt_sb_{g}", tag="xt_sb")
                xt_tiles.append(xt_sb)
            # Group 0 chunked with the W tiles so the first matmul starts
            # early; group 1 also chunked so its data drips in during the
            # group-0 tail. All loads stay on one queue in need order so
            # nothing steals HBM bandwidth from earlier-needed bytes.
            for ko in range(KO):
                nc.sync.dma_start(wt_sb[:, ko, :512], wt_d[ko][:, :512])
                nc.sync.dma_start(xt_tiles[0][:, ko, :], xt_d[0][:, ko, :])
                nc.sync.dma_start(wt_sb[:, ko, 512:], wt_d[ko][:, 512:])
            nc.scalar.dma_start(bias_sb[:], bias_d[:])
            for ko in range(KO):
                nc.sync.dma_start(xt_tiles[1][:, ko, :], xt_d[1][:, ko, :])
            for ko in range(0, KO, 2):
                nc.sync.dma_start(xt_tiles[2][:, ko:ko + 2, :],
                                  xt_d[2][:, ko:ko + 2, :])
            for ko in range(0, KO, 2):
                nc.sync.dma_start(xt_tiles[3][:, ko:ko + 2, :],
                                  xt_d[3][:, ko:ko + 2, :])

            def alloc_acc(name):
                return [
                    psum_pool.tile([P, 512], mybir.dt.float32, name=f"{name}_{h}",
                                   tag="ps_acc")
                    for h in range(NH)
                ]

            def mm(acc, xt_sb, bt, ko):
                lhsT = xt_sb[:, ko, bt * P:(bt + 1) * P]
                for h in range(NH):
                    nc.tensor.matmul(
                        acc[h][:],
                        lhsT,
                        wt_sb[:, ko, h * 512:(h + 1) * 512],
                        start=(ko == 0),
                        stop=(ko == KO - 1),
                    )

            def drain(g, bt, acc, split_store=False):
                out_sb = out_pool.tile([P, N], mybir.dt.float32, name="out_sb")
                row0 = g * GROUP_ROWS + bt * P
                for h in range(NH):
                    nc.vector.tensor_add(
                        out=out_sb[:, h * 512:(h + 1) * 512],
                        in0=acc[h][:],
                        in1=bias_sb[:, h * 512:(h + 1) * 512],
                    )
                    if split_store:
                        # store each half as soon as its drain lands so the
                        # final transfer isn't a full 512 KB on the critical
                        # path (only worth the extra issue cost at the tail)
                        nc.scalar.dma_start(
                            out_d[row0:row0 + P, h * 512:(h + 1) * 512],
                            out_sb[:, h * 512:(h + 1) * 512],
                        )
                if not split_store:
                    nc.scalar.dma_start(out_d[row0:row0 + P, :], out_sb[:])

            # Group 0: K outermost across all 4 batch tiles so each arriving
            # (wt, xt) chunk is consumed by 8 matmuls while later chunks load.
            accs0 = [alloc_acc(f"ps_g0_b{bt}") for bt in range(BT_PER_GROUP)]

            # PE warm-up: dummy matmuls on scratch data (no DMA dependency) so
            # the HAM clock gate opens during the initial load window. They
            # scribble on group 0 / bt 0's accumulator, which the real ko=0
            # matmul resets via start=True.
            for _ in range(WARMUP_MM):
                nc.tensor.matmul(
                    accs0[0][0][:, :P], warm_sb[:, :P], warm_sb[:, :P],
                    start=True, stop=True,
                )

            for ko in range(KO):
                for bt in range(BT_PER_GROUP):
                    mm(accs0[bt], xt_tiles[0], bt, ko)
                    if ko == KO - 1:
                        # drain as soon as this tile's accumulation closes so
                        # PSUM slots free up for the next group
                        drain(0, bt, accs0[bt])

            # Group 1: same K-outer shape — consumes its chunks as they land.
            accs1 = [alloc_acc(f"ps_g1_b{bt}") for bt in range(BT_PER_GROUP)]
            for ko in range(KO):
                for bt in range(BT_PER_GROUP):
                    mm(accs1[bt], xt_tiles[1], bt, ko)
                    if ko == KO - 1:
                        drain(1, bt, accs1[bt])

            # Groups 2-3: data prefetched; accumulate per batch tile.
            for g in range(2, GROUPS):
                for bt in range(BT_PER_GROUP):
                    acc = alloc_acc("ps_acc")
                    for ko in range(KO):
                        mm(acc, xt_tiles[g], bt, ko)
                    last = (g == GROUPS - 1) and (bt == BT_PER_GROUP - 1)
                    drain(g, bt, acc, split_store=last)

    nc.compile()
    return nc


def kernel(x, abcd_list, bias, _trace=False):
    x = np.ascontiguousarray(np.asarray(x, dtype=np.float32))
    bias = np.asarray(bias, dtype=np.float32)

    M = _build_dense_matrix(abcd_list).astype(np.float32)
    wt3 = np.ascontiguousarray(M.reshape(KO, P, N))       # [ko, p, n]
    bias_bc = np.ascontiguousarray(np.broadcast_to(bias[None, :], (P, N)))

    nc = _build_bass()

    in_maps = []
    for c in range(NCORES):
        xs = x[c * ROWS_PER_CORE:(c + 1) * ROWS_PER_CORE]
        # xt4[g, p, ko, b] = xs[g*512 + b, ko*128 + p]
        xt4 = np.ascontiguousarray(
            xs.reshape(GROUPS, GROUP_ROWS, KO, P).transpose(0, 3, 2, 1)
        )
        in_maps.append({"xt": xt4, "wt": wt3, "bias_bc": bias_bc})

    res = run_bass_kernel_spmd(
        nc, in_maps, core_ids=list(range(NCORES)), trace=_trace
    )
    out = np.concatenate([r["out"] for r in res.results], axis=0)
    if _trace:
        kernel.last_results = res
    return out
